# revision 38
# baseline (speedup 1.0000x reference)
"""FLAME head-model forward pass on 8 Trainium2 NeuronCores.

Pure data parallel: 128 batch elements per core, batch on the 128 SBUF
partitions, vertices on the free dimension.

  host   : O(B) prep  -- Rodrigues rotations, kinematic chain, relative
           transforms, yaw-based dynamic-landmark selection, and the final
           O(B*L) landmark gathers (tiny numpy work).
  device : O(B*V) work, pipelined in 5 vertex fragments of 1024:
           1. v_posed = [betas|1|pose_feat]^T @ [shapedirs|template|posedirs]
              (bf16 weights, fp32 PSUM accumulation, K=187 split 128+59)
           2. A[b,(c,n),v] = sum_j lbs_w[v,j] * rel_tf[b,j,c,n]  (fp32r,
              K=5 matmuls row-packed 3-at-a-time into PE row groups 0/32/64)
           3. skinning apply on DVE in bf16 (tree accumulation) with the
              final add + (v,c)-interleave on GPSIMD; PSUM drains are split
              between the Scalar and Vector engines to balance the pipeline.

Engine budget per core (cost-model timeline ~84.5us): ACT ~55us of PSUM
drains, DVE ~47us apply+drains, PE ~45us matmuls, Pool ~35us finals+DMA,
~24 MB of HBM traffic.
"""
import sys

sys.path.insert(0, "/opt/trn_rl_repo")

import numpy as np
import ml_dtypes

BF16 = ml_dtypes.bfloat16

B, V, FC, J = 1024, 5023, 9976, 5
NSHAPE, NEXP = 100, 50
PARENTS = np.array([-1, 0, 1, 1, 1])
N_CORES = 8
B_LOC = B // N_CORES  # 128

VP = 5120            # padded vertex count
KX = 187             # 150 betas + 1 const + 36 pose-feature rows
KX0 = 128            # first contraction chunk
KX1 = KX - KX0       # 59
CHUNK = 512          # LBS vertex chunk
NFRAG = 5            # pipeline fragments ("fifths")
VQ = VP // NFRAG     # 1024 vertices per fragment
CH_PER_Q = VQ // CHUNK       # 2 chunks per fragment

_STATE = {}


# ---------------------------------------------------------------- host math

def _rodrigues(rv):
    """Match reference.batch_rodrigues bit-for-bit in fp32."""
    rv = rv.astype(np.float32)
    angle = np.sqrt(((rv + np.float32(1e-8)) ** 2).sum(1, keepdims=True))
    n = rv / angle
    rx, ry, rz = n[:, 0], n[:, 1], n[:, 2]
    z = np.zeros_like(rx)
    K = np.stack([z, -rz, ry, rz, z, -rx, -ry, rx, z], 1).reshape(-1, 3, 3)
    s = np.sin(angle)[..., None]
    c = np.cos(angle)[..., None]
    I = np.eye(3, dtype=np.float32)
    return I + s * K + (np.float32(1.0) - c) * (K @ K)


def _host_prep(shape_params, expression_params, pose_params, v_template,
               shapedirs, posedirs, J_regressor, lbs_weights, neck_pose,
               eye_pose):
    f32 = np.float32
    b = shape_params.shape[0]
    betas = np.concatenate([shape_params, expression_params], 1).astype(f32)
    full_pose = np.concatenate(
        [pose_params[:, :3], np.broadcast_to(neck_pose, (b, 3)),
         pose_params[:, 3:], np.broadcast_to(eye_pose, (b, 6))], 1).astype(f32)

    rot = _rodrigues(full_pose.reshape(-1, 3)).reshape(b, J, 3, 3)
    ident = np.eye(3, dtype=f32)
    pose_feature = (rot[:, 1:] - ident).reshape(b, 36)

    # joints from betas directly:  joints = Jr@vt + (Jr@S) . betas
    Jt = (J_regressor @ v_template).astype(f32)                     # [J,3]
    JD = np.einsum("jv,vcl->jcl", J_regressor, shapedirs).astype(f32)
    joints = Jt[None] + np.einsum("bl,jcl->bjc", betas, JD)         # [B,J,3]

    rel_joints = joints.copy()
    rel_joints[:, 1:] -= joints[:, PARENTS[1:]]

    # kinematic chain of [R|t] transforms (4th row implicit [0,0,0,1])
    Rg = np.empty((b, J, 3, 3), f32)
    tg = np.empty((b, J, 3), f32)
    Rg[:, 0] = rot[:, 0]
    tg[:, 0] = rel_joints[:, 0]
    for i in range(1, J):
        p = PARENTS[i]
        Rg[:, i] = Rg[:, p] @ rot[:, i]
        tg[:, i] = np.einsum("bmn,bn->bm", Rg[:, p], rel_joints[:, i]) + tg[:, p]

    # rel_tf rows 0..2:  [Rg | tg - Rg @ joints]
    tcorr = tg - np.einsum("bjmn,bjn->bjm", Rg, joints)
    rel = np.concatenate([Rg, tcorr[..., None]], -1)                # [B,J,3,4]

    # dynamic landmark index via neck-chain yaw (fp32, mirrors reference)
    aa = full_pose.reshape(b, J, 3)[:, [1, 0]]
    rch = _rodrigues(aa.reshape(-1, 3)).reshape(b, 2, 3, 3)
    rel_rot = rch[:, 1] @ rch[:, 0]
    sy = np.sqrt(rel_rot[:, 0, 0] ** 2 + rel_rot[:, 1, 0] ** 2)
    yang = np.arctan2(-rel_rot[:, 2, 0], sy).astype(f32) * f32(180.0 / np.pi)
    y = np.round(np.minimum(yang, f32(39.0))).astype(np.int32)
    neg = y < 0
    big = y < -39
    y = np.where(neg, np.where(big, 78, 39 - y), y)

    # device-side constant matrices
    # Smat [187, 3*VP] with columns ordered n-outer: col = n*VP + v
    # (padded to VP per plane: fp32r matmuls need even free counts)
    Smat = np.zeros((KX, 3 * VP), f32)
    Sv = Smat.reshape(KX, 3, VP)[:, :, :V]
    Sv[:150] = shapedirs.transpose(2, 1, 0)
    Sv[150] = v_template.T
    Sv[151:] = posedirs.reshape(36, V, 3).transpose(0, 2, 1)

    xaug = np.empty((B, KX), f32)
    xaug[:, :150] = betas
    xaug[:, 150] = 1.0
    xaug[:, 151:] = pose_feature

    # relT [60, B]: row = (c*4+n)*5 + j
    relT = rel.transpose(2, 3, 1, 0).reshape(60, b)                 # (c,n,j,b)
    relT = np.ascontiguousarray(relT.astype(f32))
    # rel3 [4, 128, B]: tile g holds cn=3g+t at partitions 32t+j (row-packing)
    rel3 = np.zeros((4, 128, b), f32)
    for g in range(4):
        for t in range(3):
            rel3[g, 32 * t:32 * t + 5] = relT[(3 * g + t) * 5:(3 * g + t) * 5 + 5]

    # three stacked copies: row-packed matmuls read the moving operand at
    # base partitions 0/32/64, so each row-group gets its own parallel load
    WTp = np.zeros((3 * J, VP), f32)
    for t in range(3):
        WTp[t * J:(t + 1) * J, :V] = lbs_weights.T

    return xaug, Smat, relT, rel3, WTp, y


# ------------------------------------------------------------- device build

def _build_device():
    import concourse.bacc as bacc
    import concourse.tile as tile
    from concourse import mybir

    R = mybir.dt.float32r
    F = mybir.dt.float32
    H = mybir.dt.bfloat16

    nc = bacc.Bacc("TRN2", target_bir_lowering=False, debug=False)
    d_x0 = nc.dram_tensor("x0", [KX0, B_LOC], H, kind="ExternalInput").ap()
    d_x1 = nc.dram_tensor("x1", [KX1, B_LOC], H, kind="ExternalInput").ap()
    d_s0 = nc.dram_tensor("s0", [KX0, 3 * VP], H, kind="ExternalInput").ap()
    d_s1 = nc.dram_tensor("s1", [KX1, 3 * VP], H, kind="ExternalInput").ap()
    d_rel = nc.dram_tensor("rel3", [4, 128, B_LOC], F, kind="ExternalInput").ap()
    d_wt = nc.dram_tensor("wt", [3 * J, VP], F, kind="ExternalInput").ap()
    d_out = nc.dram_tensor("verts", [B_LOC, 3 * V], F, kind="ExternalOutput").ap()

    with tile.TileContext(nc) as tc:
        with tc.tile_pool(name="const", bufs=1) as cpool, \
             tc.tile_pool(name="sstream", bufs=2) as spool, \
             tc.tile_pool(name="vp", bufs=1) as vpool, \
             tc.tile_pool(name="abuf", bufs=2) as apool, \
             tc.tile_pool(name="vout", bufs=2) as opool, \
             tc.tile_pool(name="tmp", bufs=2) as tpool, \
             tc.tile_pool(name="ps2", bufs=1, space="PSUM") as ps2, \
             tc.tile_pool(name="ps3", bufs=2, space="PSUM") as ps3:

            # ---- constants: 4 packed rel tiles (cn=3g+t at partitions 32t+j)
            wt = cpool.tile([69, VP], R, tag="wt")
            nc.sync.dma_start(wt[0:J, :], d_wt[0:J, :].bitcast(R))
            nc.gpsimd.dma_start(wt[32:32 + J, :], d_wt[J:2 * J, :].bitcast(R))
            nc.scalar.dma_start(wt[64:64 + J, :], d_wt[2 * J:3 * J, :].bitcast(R))
            x0 = cpool.tile([KX0, B_LOC], H, tag="x0")
            x1 = cpool.tile([KX1, B_LOC], H, tag="x1")
            nc.gpsimd.dma_start(x0[:], d_x0[:])
            nc.gpsimd.dma_start(x1[:], d_x1[:])
            rel3 = []
            for g in range(4):
                t = cpool.tile([128, B_LOC], R, tag=f"rel3g{g}", name=f"rel3g{g}")
                eng = nc.scalar if g < 2 else nc.gpsimd
                eng.dma_start(t[:], d_rel[g].bitcast(R))
                rel3.append(t)

            vps = [[None] * 3 for _ in range(NFRAG)]

            for q in range(NFRAG):
                # ---- LBS transform matmuls first (only need tiny consts).
                # Quad qd covers planes {3g..3g+2}; the 3 K=5 matmuls sit in
                # row-groups 0/32/64 of the PE array and run concurrently.
                aq = apool.tile([B_LOC, 12 * VQ], H, tag="abuf")
                ap12 = aq[:].rearrange("p (t v) -> p t v", t=12)
                for k in range(CH_PER_Q):
                    v0 = (q * CH_PER_Q + k) * CHUNK
                    for g in range(4):
                        ps = ps3.tile([B_LOC, 3 * CHUNK], F, tag="ps3")
                        for t in range(3):
                            nc.tensor.matmul(
                                ps[:, t * CHUNK:(t + 1) * CHUNK],
                                rel3[g][32 * t:32 * t + 5, :],
                                wt[32 * t:32 * t + 5, v0:v0 + CHUNK],
                                start=True, stop=True)
                        dst = ap12[:, 3 * g:3 * g + 3,
                                   k * CHUNK:(k + 1) * CHUNK]
                        # f0 is latency-critical: alternate DVE/ACT evenly;
                        # steady state: DVE ~1.5 of 8 quads, rest ACT
                        if q == 0:
                            if g % 2 == 1:
                                nc.vector.tensor_copy(dst, ps[:])
                            else:
                                nc.scalar.copy(dst, ps[:])
                        elif g == 3 and k == 0:
                            nc.vector.tensor_copy(dst, ps[:])
                        elif g == 3 and k == 1:
                            nc.vector.tensor_copy(dst[:, 0:2, :],
                                                  ps[:, :2 * CHUNK])
                            nc.scalar.copy(dst[:, 2:3, :],
                                           ps[:, 2 * CHUNK:])
                        else:
                            nc.scalar.copy(dst, ps[:])

                # ---- blendshape+pose matmul for this fragment's vp planes
                for n in range(3):
                    s0t = spool.tile([KX0, VQ], H, tag="s0")
                    s1t = spool.tile([KX1, VQ], H, tag="s1")
                    base = n * VP + q * VQ
                    nc.sync.dma_start(s0t[:], d_s0[:, base:base + VQ])
                    nc.sync.dma_start(s1t[:], d_s1[:, base:base + VQ])
                    vpt = vpool.tile([B_LOC, VQ], H, tag=f"vp{n}q{q}",
                                     name=f"vp{n}q{q}")
                    ps = ps2.tile([B_LOC, VQ], F, tag="ps2")
                    for sub in range(0, VQ, 512):
                        nc.tensor.matmul(ps[:, sub:sub + 512], x0[:],
                                         s0t[:, sub:sub + 512],
                                         start=True, stop=False)
                        nc.tensor.matmul(ps[:, sub:sub + 512], x1[:],
                                         s1t[:, sub:sub + 512],
                                         start=False, stop=True)
                    nc.scalar.copy(vpt[:], ps[:])
                    vps[q][n] = vpt

                # ---- skinning apply (tree accumulation, flat plane APs)
                vo = opool.tile([B_LOC, 3 * VQ], F, tag="vout")
                voc = vo[:].rearrange("p (v c) -> p v c", c=3)
                vq = [vps[q][n][:] for n in range(3)]
                # last fragment: compute only the real (padded-to-even) width
                VA = min(VQ, ((3 * V - q * 3 * VQ) // 3 + 1) // 2 * 2)
                t01s, t23s = [], []
                for c in range(3):
                    pa = tpool.tile([B_LOC, VQ], H, tag="pa")
                    pb = tpool.tile([B_LOC, VQ], H, tag="pb")
                    t01 = tpool.tile([B_LOC, VQ], H, tag=f"t01{c}",
                                     name=f"t01c{c}")
                    t23 = tpool.tile([B_LOC, VQ], H, tag=f"t23{c}",
                                     name=f"t23c{c}")
                    nc.vector.tensor_mul(pa[:, :VA], ap12[:, 4 * c + 0, :VA],
                                         vq[0][:, :VA])
                    nc.vector.tensor_mul(pb[:, :VA], ap12[:, 4 * c + 1, :VA],
                                         vq[1][:, :VA])
                    nc.vector.tensor_add(t01[:, :VA], pa[:, :VA], pb[:, :VA])
                    p2 = tpool.tile([B_LOC, VQ], H, tag="pa", name="p2")
                    nc.vector.tensor_mul(p2[:, :VA], ap12[:, 4 * c + 2, :VA],
                                         vq[2][:, :VA])
                    nc.vector.tensor_add(t23[:, :VA], p2[:, :VA],
                                         ap12[:, 4 * c + 3, :VA])
                    t01s.append(t01)
                    t23s.append(t23)

                # final adds + (v,c) interleave on the otherwise-idle gpsimd,
                # by half-fragment so the store starts before the last final
                HVQ = VQ // 2
                for h in range(2):
                    vsl = slice(h * HVQ, min((h + 1) * HVQ, VA))
                    for c in range(3):
                        feng = nc.vector if (q == NFRAG - 1 and h == 0) \
                            else nc.gpsimd
                        feng.tensor_add(voc[:, vsl, c],
                                        t01s[c][:, vsl], t23s[c][:, vsl])
                    vbase = q * 3 * VQ + h * 3 * HVQ
                    vwid = min(3 * HVQ, max(0, 3 * V - vbase))
                    if vwid > 0:
                        nc.gpsimd.dma_start(
                            d_out[:, vbase:vbase + vwid],
                            vo[:, h * 3 * HVQ:h * 3 * HVQ + vwid])

    nc.compile()
    return nc


def _get_nc():
    if "nc" not in _STATE:
        _STATE["nc"] = _build_device()
    return _STATE["nc"]


# -------------------------------------------------------------------- kernel

def kernel(shape_params, expression_params, pose_params, v_template, shapedirs,
           posedirs, J_regressor, lbs_weights, neck_pose, eye_pose, faces,
           lmk_faces_idx, lmk_bary_coords, dynamic_lmk_faces_idx,
           dynamic_lmk_bary_coords, full_lmk_faces_idx, full_lmk_bary_coords):
    from concourse.bass_utils import run_bass_kernel_spmd

    f32 = np.float32
    inputs = [np.asarray(a) for a in
              (shape_params, expression_params, pose_params, v_template,
               shapedirs, posedirs, J_regressor, lbs_weights, neck_pose,
               eye_pose)]
    xaug, Smat, relT, rel3, WTp, y = _host_prep(*inputs)

    s0 = np.ascontiguousarray(Smat[:KX0]).astype(BF16)
    s1 = np.ascontiguousarray(Smat[KX0:]).astype(BF16)
    xT = np.ascontiguousarray(xaug.T).astype(BF16)     # [187, B]

    core_ids = list(range(N_CORES))
    in_maps = []
    for c in core_ids:
        sl = slice(c * B_LOC, (c + 1) * B_LOC)
        in_maps.append({
            "x0": np.ascontiguousarray(xT[:KX0, sl]),
            "x1": np.ascontiguousarray(xT[KX0:, sl]),
            "s0": s0,
            "s1": s1,
            "rel3": np.ascontiguousarray(rel3[:, :, sl]),
            "wt": WTp,
        })

    nc = _get_nc()
    res = run_bass_kernel_spmd(nc, in_maps, core_ids)
    _STATE["last_result"] = res

    verts = np.concatenate([res.results[c]["verts"] for c in core_ids], 0)
    vertices = verts.reshape(B, V, 3).astype(f32, copy=False)

    # ---- landmarks on host (tiny gathers)
    faces = np.asarray(faces)
    lmk_faces_idx = np.asarray(lmk_faces_idx)
    lmk_bary_coords = np.asarray(lmk_bary_coords).astype(f32)
    dynamic_lmk_faces_idx = np.asarray(dynamic_lmk_faces_idx)
    dynamic_lmk_bary_coords = np.asarray(dynamic_lmk_bary_coords).astype(f32)
    full_lmk_faces_idx = np.asarray(full_lmk_faces_idx)
    full_lmk_bary_coords = np.asarray(full_lmk_bary_coords).astype(f32)

    bidx = np.arange(B)[:, None, None]

    dyn_tri = faces[dynamic_lmk_faces_idx[y]]            # [B,17,3]
    dyn_bary = dynamic_lmk_bary_coords[y]                # [B,17,3]
    lv = vertices[bidx, dyn_tri]                         # [B,17,3,3]
    lm_dyn = np.einsum("blfc,blf->blc", lv, dyn_bary)

    st_tri = faces[lmk_faces_idx]                        # [51,3]
    lv = vertices[:, st_tri]                             # [B,51,3,3]
    lm_st = np.einsum("blfc,lf->blc", lv, lmk_bary_coords)

    landmarks2d = np.concatenate([lm_dyn, lm_st], 1).astype(f32, copy=False)

    fl_tri = faces[full_lmk_faces_idx]                   # [68,3]
    lv = vertices[:, fl_tri]                             # [B,68,3,3]
    landmarks3d = np.einsum("blfc,lf->blc", lv,
                            full_lmk_bary_coords).astype(f32, copy=False)

    return vertices, landmarks2d, landmarks3d


# revision 39
# speedup vs baseline: 1.0279x; 1.0279x over previous
"""FLAME head-model forward pass on 8 Trainium2 NeuronCores.

Pure data parallel: 128 batch elements per core, batch on the 128 SBUF
partitions, vertices on the free dimension.

  host   : O(B) prep  -- Rodrigues rotations, kinematic chain, relative
           transforms, yaw-based dynamic-landmark selection, and the final
           O(B*L) landmark gathers (tiny numpy work).
  device : O(B*V) work, pipelined in 5 vertex fragments of 1024:
           1. v_posed = [betas|1|pose_feat]^T @ [shapedirs|template|posedirs]
              (bf16 weights, fp32 PSUM accumulation, K=187 split 128+59)
           2. A[b,(c,n),v] = sum_j lbs_w[v,j] * rel_tf[b,j,c,n]  (fp32r,
              K=5 matmuls row-packed 3-at-a-time into PE row groups 0/32/64)
           3. skinning apply on DVE in bf16 (tree accumulation) with the
              final add + (v,c)-interleave on GPSIMD; PSUM drains are split
              between the Scalar and Vector engines to balance the pipeline.

Engine budget per core (cost-model timeline ~84.5us): ACT ~55us of PSUM
drains, DVE ~47us apply+drains, PE ~45us matmuls, Pool ~35us finals+DMA,
~24 MB of HBM traffic.
"""
import sys

sys.path.insert(0, "/opt/trn_rl_repo")

import numpy as np
import ml_dtypes

BF16 = ml_dtypes.bfloat16

B, V, FC, J = 1024, 5023, 9976, 5
NSHAPE, NEXP = 100, 50
PARENTS = np.array([-1, 0, 1, 1, 1])
N_CORES = 8
B_LOC = B // N_CORES  # 128

VP = 5120            # padded vertex count
KX = 187             # 150 betas + 1 const + 36 pose-feature rows
KX0 = 128            # first contraction chunk
KX1 = KX - KX0       # 59
CHUNK = 512          # LBS vertex chunk
NFRAG = 5            # pipeline fragments ("fifths")
VQ = VP // NFRAG     # 1024 vertices per fragment
CH_PER_Q = VQ // CHUNK       # 2 chunks per fragment

_STATE = {}


# ---------------------------------------------------------------- host math

def _rodrigues(rv):
    """Match reference.batch_rodrigues bit-for-bit in fp32."""
    rv = rv.astype(np.float32)
    angle = np.sqrt(((rv + np.float32(1e-8)) ** 2).sum(1, keepdims=True))
    n = rv / angle
    rx, ry, rz = n[:, 0], n[:, 1], n[:, 2]
    z = np.zeros_like(rx)
    K = np.stack([z, -rz, ry, rz, z, -rx, -ry, rx, z], 1).reshape(-1, 3, 3)
    s = np.sin(angle)[..., None]
    c = np.cos(angle)[..., None]
    I = np.eye(3, dtype=np.float32)
    return I + s * K + (np.float32(1.0) - c) * (K @ K)


def _host_prep(shape_params, expression_params, pose_params, v_template,
               shapedirs, posedirs, J_regressor, lbs_weights, neck_pose,
               eye_pose):
    f32 = np.float32
    b = shape_params.shape[0]
    betas = np.concatenate([shape_params, expression_params], 1).astype(f32)
    full_pose = np.concatenate(
        [pose_params[:, :3], np.broadcast_to(neck_pose, (b, 3)),
         pose_params[:, 3:], np.broadcast_to(eye_pose, (b, 6))], 1).astype(f32)

    rot = _rodrigues(full_pose.reshape(-1, 3)).reshape(b, J, 3, 3)
    ident = np.eye(3, dtype=f32)
    pose_feature = (rot[:, 1:] - ident).reshape(b, 36)

    # joints from betas directly:  joints = Jr@vt + (Jr@S) . betas
    Jt = (J_regressor @ v_template).astype(f32)                     # [J,3]
    JD = np.einsum("jv,vcl->jcl", J_regressor, shapedirs).astype(f32)
    joints = Jt[None] + np.einsum("bl,jcl->bjc", betas, JD)         # [B,J,3]

    rel_joints = joints.copy()
    rel_joints[:, 1:] -= joints[:, PARENTS[1:]]

    # kinematic chain of [R|t] transforms (4th row implicit [0,0,0,1])
    Rg = np.empty((b, J, 3, 3), f32)
    tg = np.empty((b, J, 3), f32)
    Rg[:, 0] = rot[:, 0]
    tg[:, 0] = rel_joints[:, 0]
    for i in range(1, J):
        p = PARENTS[i]
        Rg[:, i] = Rg[:, p] @ rot[:, i]
        tg[:, i] = np.einsum("bmn,bn->bm", Rg[:, p], rel_joints[:, i]) + tg[:, p]

    # rel_tf rows 0..2:  [Rg | tg - Rg @ joints]
    tcorr = tg - np.einsum("bjmn,bjn->bjm", Rg, joints)
    rel = np.concatenate([Rg, tcorr[..., None]], -1)                # [B,J,3,4]

    # dynamic landmark index via neck-chain yaw (fp32, mirrors reference)
    aa = full_pose.reshape(b, J, 3)[:, [1, 0]]
    rch = _rodrigues(aa.reshape(-1, 3)).reshape(b, 2, 3, 3)
    rel_rot = rch[:, 1] @ rch[:, 0]
    sy = np.sqrt(rel_rot[:, 0, 0] ** 2 + rel_rot[:, 1, 0] ** 2)
    yang = np.arctan2(-rel_rot[:, 2, 0], sy).astype(f32) * f32(180.0 / np.pi)
    y = np.round(np.minimum(yang, f32(39.0))).astype(np.int32)
    neg = y < 0
    big = y < -39
    y = np.where(neg, np.where(big, 78, 39 - y), y)

    # device-side constant matrices
    # Smat [187, 3*VP] with columns ordered n-outer: col = n*VP + v
    # (padded to VP per plane: fp32r matmuls need even free counts)
    Smat = np.zeros((KX, 3 * VP), f32)
    Sv = Smat.reshape(KX, 3, VP)[:, :, :V]
    Sv[:150] = shapedirs.transpose(2, 1, 0)
    Sv[150] = v_template.T
    Sv[151:] = posedirs.reshape(36, V, 3).transpose(0, 2, 1)

    xaug = np.empty((B, KX), f32)
    xaug[:, :150] = betas
    xaug[:, 150] = 1.0
    xaug[:, 151:] = pose_feature

    # relT [60, B]: row = (c*4+n)*5 + j
    relT = rel.transpose(2, 3, 1, 0).reshape(60, b)                 # (c,n,j,b)
    relT = np.ascontiguousarray(relT.astype(f32))
    # rel3 [4, 128, B]: tile g holds cn=3g+t at partitions 32t+j (row-packing)
    rel3 = np.zeros((4, 128, b), f32)
    for g in range(4):
        for t in range(3):
            rel3[g, 32 * t:32 * t + 5] = relT[(3 * g + t) * 5:(3 * g + t) * 5 + 5]

    # three stacked copies: row-packed matmuls read the moving operand at
    # base partitions 0/32/64, so each row-group gets its own parallel load
    WTp = np.zeros((3 * J, VP), f32)
    for t in range(3):
        WTp[t * J:(t + 1) * J, :V] = lbs_weights.T

    return xaug, Smat, relT, rel3, WTp, y


# ------------------------------------------------------------- device build

def _build_device():
    import concourse.bacc as bacc
    import concourse.tile as tile
    from concourse import mybir

    R = mybir.dt.float32r
    F = mybir.dt.float32
    H = mybir.dt.bfloat16

    nc = bacc.Bacc("TRN2", target_bir_lowering=False, debug=False)
    d_x0 = nc.dram_tensor("x0", [KX0, B_LOC], H, kind="ExternalInput").ap()
    d_x1 = nc.dram_tensor("x1", [KX1, B_LOC], H, kind="ExternalInput").ap()
    d_s0 = nc.dram_tensor("s0", [KX0, 3 * VP], H, kind="ExternalInput").ap()
    d_s1 = nc.dram_tensor("s1", [KX1, 3 * VP], H, kind="ExternalInput").ap()
    d_rel = nc.dram_tensor("rel3", [4, 128, B_LOC], F, kind="ExternalInput").ap()
    d_wt = nc.dram_tensor("wt", [3 * J, VP], F, kind="ExternalInput").ap()
    d_out = nc.dram_tensor("verts", [B_LOC, 3 * V], F, kind="ExternalOutput").ap()

    with tile.TileContext(nc) as tc:
        with tc.tile_pool(name="const", bufs=1) as cpool, \
             tc.tile_pool(name="sstream", bufs=2) as spool, \
             tc.tile_pool(name="vp", bufs=1) as vpool, \
             tc.tile_pool(name="abuf", bufs=2) as apool, \
             tc.tile_pool(name="vout", bufs=2) as opool, \
             tc.tile_pool(name="tmp", bufs=2) as tpool, \
             tc.tile_pool(name="ps2", bufs=1, space="PSUM") as ps2, \
             tc.tile_pool(name="ps3", bufs=2, space="PSUM") as ps3:

            # ---- constants: 4 packed rel tiles (cn=3g+t at partitions 32t+j)
            wt = cpool.tile([69, VP], R, tag="wt")
            nc.scalar.dma_start(wt[0:J, :], d_wt[0:J, :].bitcast(R))
            nc.gpsimd.dma_start(wt[32:32 + J, :], d_wt[J:2 * J, :].bitcast(R))
            nc.scalar.dma_start(wt[64:64 + J, :], d_wt[2 * J:3 * J, :].bitcast(R))
            x0 = cpool.tile([KX0, B_LOC], H, tag="x0")
            x1 = cpool.tile([KX1, B_LOC], H, tag="x1")
            nc.gpsimd.dma_start(x0[:], d_x0[:])
            nc.gpsimd.dma_start(x1[:], d_x1[:])
            rel3 = []
            for g in range(4):
                t = cpool.tile([128, B_LOC], R, tag=f"rel3g{g}", name=f"rel3g{g}")
                eng = nc.scalar if g < 2 else nc.gpsimd
                eng.dma_start(t[:], d_rel[g].bitcast(R))
                rel3.append(t)

            vps = [[None] * 3 for _ in range(NFRAG)]

            for q in range(NFRAG):
                # ---- LBS transform matmuls first (only need tiny consts).
                # Quad qd covers planes {3g..3g+2}; the 3 K=5 matmuls sit in
                # row-groups 0/32/64 of the PE array and run concurrently.
                aq = apool.tile([B_LOC, 12 * VQ], H, tag="abuf")
                ap12 = aq[:].rearrange("p (t v) -> p t v", t=12)
                for k in range(CH_PER_Q):
                    v0 = (q * CH_PER_Q + k) * CHUNK
                    for g in range(4):
                        ps = ps3.tile([B_LOC, 3 * CHUNK], F, tag="ps3")
                        for t in range(3):
                            nc.tensor.matmul(
                                ps[:, t * CHUNK:(t + 1) * CHUNK],
                                rel3[g][32 * t:32 * t + 5, :],
                                wt[32 * t:32 * t + 5, v0:v0 + CHUNK],
                                start=True, stop=True)
                        dst = ap12[:, 3 * g:3 * g + 3,
                                   k * CHUNK:(k + 1) * CHUNK]
                        # f0 is latency-critical: alternate DVE/ACT evenly;
                        # steady state: DVE ~1.5 of 8 quads, rest ACT
                        if q == 0:
                            if g % 2 == 1:
                                nc.vector.tensor_copy(dst, ps[:])
                            else:
                                nc.scalar.copy(dst, ps[:])
                        elif g == 3 and k == 0:
                            nc.vector.tensor_copy(dst, ps[:])
                        elif g == 3 and k == 1:
                            nc.vector.tensor_copy(dst[:, 0:2, :],
                                                  ps[:, :2 * CHUNK])
                            nc.scalar.copy(dst[:, 2:3, :],
                                           ps[:, 2 * CHUNK:])
                        else:
                            nc.scalar.copy(dst, ps[:])

                # ---- blendshape+pose matmul for this fragment's vp planes
                for n in range(3):
                    s0t = spool.tile([KX0, VQ], H, tag="s0")
                    s1t = spool.tile([KX1, VQ], H, tag="s1")
                    base = n * VP + q * VQ
                    nc.sync.dma_start(s0t[:], d_s0[:, base:base + VQ])
                    nc.sync.dma_start(s1t[:], d_s1[:, base:base + VQ])
                    vpt = vpool.tile([B_LOC, VQ], H, tag=f"vp{n}q{q}",
                                     name=f"vp{n}q{q}")
                    ps = ps2.tile([B_LOC, VQ], F, tag="ps2")
                    for sub in range(0, VQ, 512):
                        nc.tensor.matmul(ps[:, sub:sub + 512], x0[:],
                                         s0t[:, sub:sub + 512],
                                         start=True, stop=False)
                        nc.tensor.matmul(ps[:, sub:sub + 512], x1[:],
                                         s1t[:, sub:sub + 512],
                                         start=False, stop=True)
                    nc.scalar.copy(vpt[:], ps[:])
                    vps[q][n] = vpt

                # ---- skinning apply (tree accumulation, flat plane APs)
                vo = opool.tile([B_LOC, 3 * VQ], F, tag="vout")
                voc = vo[:].rearrange("p (v c) -> p v c", c=3)
                vq = [vps[q][n][:] for n in range(3)]
                # last fragment: compute only the real (padded-to-even) width
                VA = min(VQ, ((3 * V - q * 3 * VQ) // 3 + 1) // 2 * 2)
                t01s, t23s = [], []
                for c in range(3):
                    pa = tpool.tile([B_LOC, VQ], H, tag="pa")
                    pb = tpool.tile([B_LOC, VQ], H, tag="pb")
                    t01 = tpool.tile([B_LOC, VQ], H, tag=f"t01{c}",
                                     name=f"t01c{c}")
                    t23 = tpool.tile([B_LOC, VQ], H, tag=f"t23{c}",
                                     name=f"t23c{c}")
                    nc.vector.tensor_mul(pa[:, :VA], ap12[:, 4 * c + 0, :VA],
                                         vq[0][:, :VA])
                    nc.vector.tensor_mul(pb[:, :VA], ap12[:, 4 * c + 1, :VA],
                                         vq[1][:, :VA])
                    nc.vector.tensor_add(t01[:, :VA], pa[:, :VA], pb[:, :VA])
                    p2 = tpool.tile([B_LOC, VQ], H, tag="pa", name="p2")
                    nc.vector.tensor_mul(p2[:, :VA], ap12[:, 4 * c + 2, :VA],
                                         vq[2][:, :VA])
                    nc.vector.tensor_add(t23[:, :VA], p2[:, :VA],
                                         ap12[:, 4 * c + 3, :VA])
                    t01s.append(t01)
                    t23s.append(t23)

                # final adds + (v,c) interleave on the otherwise-idle gpsimd,
                # by half-fragment so the store starts before the last final
                HVQ = VQ // 2
                for h in range(2):
                    vsl = slice(h * HVQ, min((h + 1) * HVQ, VA))
                    for c in range(3):
                        feng = nc.vector if (q == NFRAG - 1 and h == 0) \
                            else nc.gpsimd
                        feng.tensor_add(voc[:, vsl, c],
                                        t01s[c][:, vsl], t23s[c][:, vsl])
                    vbase = q * 3 * VQ + h * 3 * HVQ
                    vwid = min(3 * HVQ, max(0, 3 * V - vbase))
                    if vwid > 0:
                        nc.gpsimd.dma_start(
                            d_out[:, vbase:vbase + vwid],
                            vo[:, h * 3 * HVQ:h * 3 * HVQ + vwid])

    nc.compile()
    return nc


def _get_nc():
    if "nc" not in _STATE:
        _STATE["nc"] = _build_device()
    return _STATE["nc"]


# -------------------------------------------------------------------- kernel

def kernel(shape_params, expression_params, pose_params, v_template, shapedirs,
           posedirs, J_regressor, lbs_weights, neck_pose, eye_pose, faces,
           lmk_faces_idx, lmk_bary_coords, dynamic_lmk_faces_idx,
           dynamic_lmk_bary_coords, full_lmk_faces_idx, full_lmk_bary_coords):
    from concourse.bass_utils import run_bass_kernel_spmd

    f32 = np.float32
    inputs = [np.asarray(a) for a in
              (shape_params, expression_params, pose_params, v_template,
               shapedirs, posedirs, J_regressor, lbs_weights, neck_pose,
               eye_pose)]
    xaug, Smat, relT, rel3, WTp, y = _host_prep(*inputs)

    s0 = np.ascontiguousarray(Smat[:KX0]).astype(BF16)
    s1 = np.ascontiguousarray(Smat[KX0:]).astype(BF16)
    xT = np.ascontiguousarray(xaug.T).astype(BF16)     # [187, B]

    core_ids = list(range(N_CORES))
    in_maps = []
    for c in core_ids:
        sl = slice(c * B_LOC, (c + 1) * B_LOC)
        in_maps.append({
            "x0": np.ascontiguousarray(xT[:KX0, sl]),
            "x1": np.ascontiguousarray(xT[KX0:, sl]),
            "s0": s0,
            "s1": s1,
            "rel3": np.ascontiguousarray(rel3[:, :, sl]),
            "wt": WTp,
        })

    nc = _get_nc()
    res = run_bass_kernel_spmd(nc, in_maps, core_ids)
    _STATE["last_result"] = res

    verts = np.concatenate([res.results[c]["verts"] for c in core_ids], 0)
    vertices = verts.reshape(B, V, 3).astype(f32, copy=False)

    # ---- landmarks on host (tiny gathers)
    faces = np.asarray(faces)
    lmk_faces_idx = np.asarray(lmk_faces_idx)
    lmk_bary_coords = np.asarray(lmk_bary_coords).astype(f32)
    dynamic_lmk_faces_idx = np.asarray(dynamic_lmk_faces_idx)
    dynamic_lmk_bary_coords = np.asarray(dynamic_lmk_bary_coords).astype(f32)
    full_lmk_faces_idx = np.asarray(full_lmk_faces_idx)
    full_lmk_bary_coords = np.asarray(full_lmk_bary_coords).astype(f32)

    bidx = np.arange(B)[:, None, None]

    dyn_tri = faces[dynamic_lmk_faces_idx[y]]            # [B,17,3]
    dyn_bary = dynamic_lmk_bary_coords[y]                # [B,17,3]
    lv = vertices[bidx, dyn_tri]                         # [B,17,3,3]
    lm_dyn = np.einsum("blfc,blf->blc", lv, dyn_bary)

    st_tri = faces[lmk_faces_idx]                        # [51,3]
    lv = vertices[:, st_tri]                             # [B,51,3,3]
    lm_st = np.einsum("blfc,lf->blc", lv, lmk_bary_coords)

    landmarks2d = np.concatenate([lm_dyn, lm_st], 1).astype(f32, copy=False)

    fl_tri = faces[full_lmk_faces_idx]                   # [68,3]
    lv = vertices[:, fl_tri]                             # [B,68,3,3]
    landmarks3d = np.einsum("blfc,lf->blc", lv,
                            full_lmk_bary_coords).astype(f32, copy=False)

    return vertices, landmarks2d, landmarks3d


# revision 40
# speedup vs baseline: 1.0282x; 1.0003x over previous
"""FLAME head-model forward pass on 8 Trainium2 NeuronCores.

Pure data parallel: 128 batch elements per core, batch on the 128 SBUF
partitions, vertices on the free dimension.

  host   : O(B) prep  -- Rodrigues rotations, kinematic chain, relative
           transforms, yaw-based dynamic-landmark selection, and the final
           O(B*L) landmark gathers (tiny numpy work).
  device : O(B*V) work, pipelined in 5 vertex fragments of 1024:
           1. v_posed = [betas|1|pose_feat]^T @ [shapedirs|template|posedirs]
              (bf16 weights, fp32 PSUM accumulation, K=187 split 128+59)
           2. A[b,(c,n),v] = sum_j lbs_w[v,j] * rel_tf[b,j,c,n]  (fp32r,
              K=5 matmuls row-packed 3-at-a-time into PE row groups 0/32/64)
           3. skinning apply on DVE in bf16 (tree accumulation) with the
              final add + (v,c)-interleave on GPSIMD; PSUM drains are split
              between the Scalar and Vector engines to balance the pipeline.

Engine budget per core (cost-model timeline ~84.5us): ACT ~55us of PSUM
drains, DVE ~47us apply+drains, PE ~45us matmuls, Pool ~35us finals+DMA,
~24 MB of HBM traffic.
"""
import sys

sys.path.insert(0, "/opt/trn_rl_repo")

import numpy as np
import ml_dtypes

BF16 = ml_dtypes.bfloat16

B, V, FC, J = 1024, 5023, 9976, 5
NSHAPE, NEXP = 100, 50
PARENTS = np.array([-1, 0, 1, 1, 1])
N_CORES = 8
B_LOC = B // N_CORES  # 128

VP = 5120            # padded vertex count
KX = 187             # 150 betas + 1 const + 36 pose-feature rows
KX0 = 128            # first contraction chunk
KX1 = KX - KX0       # 59
CHUNK = 512          # LBS vertex chunk
NFRAG = 5            # pipeline fragments ("fifths")
VQ = VP // NFRAG     # 1024 vertices per fragment
CH_PER_Q = VQ // CHUNK       # 2 chunks per fragment

_STATE = {}


# ---------------------------------------------------------------- host math

def _rodrigues(rv):
    """Match reference.batch_rodrigues bit-for-bit in fp32."""
    rv = rv.astype(np.float32)
    angle = np.sqrt(((rv + np.float32(1e-8)) ** 2).sum(1, keepdims=True))
    n = rv / angle
    rx, ry, rz = n[:, 0], n[:, 1], n[:, 2]
    z = np.zeros_like(rx)
    K = np.stack([z, -rz, ry, rz, z, -rx, -ry, rx, z], 1).reshape(-1, 3, 3)
    s = np.sin(angle)[..., None]
    c = np.cos(angle)[..., None]
    I = np.eye(3, dtype=np.float32)
    return I + s * K + (np.float32(1.0) - c) * (K @ K)


def _host_prep(shape_params, expression_params, pose_params, v_template,
               shapedirs, posedirs, J_regressor, lbs_weights, neck_pose,
               eye_pose):
    f32 = np.float32
    b = shape_params.shape[0]
    betas = np.concatenate([shape_params, expression_params], 1).astype(f32)
    full_pose = np.concatenate(
        [pose_params[:, :3], np.broadcast_to(neck_pose, (b, 3)),
         pose_params[:, 3:], np.broadcast_to(eye_pose, (b, 6))], 1).astype(f32)

    rot = _rodrigues(full_pose.reshape(-1, 3)).reshape(b, J, 3, 3)
    ident = np.eye(3, dtype=f32)
    pose_feature = (rot[:, 1:] - ident).reshape(b, 36)

    # joints from betas directly:  joints = Jr@vt + (Jr@S) . betas
    Jt = (J_regressor @ v_template).astype(f32)                     # [J,3]
    JD = np.einsum("jv,vcl->jcl", J_regressor, shapedirs).astype(f32)
    joints = Jt[None] + np.einsum("bl,jcl->bjc", betas, JD)         # [B,J,3]

    rel_joints = joints.copy()
    rel_joints[:, 1:] -= joints[:, PARENTS[1:]]

    # kinematic chain of [R|t] transforms (4th row implicit [0,0,0,1])
    Rg = np.empty((b, J, 3, 3), f32)
    tg = np.empty((b, J, 3), f32)
    Rg[:, 0] = rot[:, 0]
    tg[:, 0] = rel_joints[:, 0]
    for i in range(1, J):
        p = PARENTS[i]
        Rg[:, i] = Rg[:, p] @ rot[:, i]
        tg[:, i] = np.einsum("bmn,bn->bm", Rg[:, p], rel_joints[:, i]) + tg[:, p]

    # rel_tf rows 0..2:  [Rg | tg - Rg @ joints]
    tcorr = tg - np.einsum("bjmn,bjn->bjm", Rg, joints)
    rel = np.concatenate([Rg, tcorr[..., None]], -1)                # [B,J,3,4]

    # dynamic landmark index via neck-chain yaw (fp32, mirrors reference)
    aa = full_pose.reshape(b, J, 3)[:, [1, 0]]
    rch = _rodrigues(aa.reshape(-1, 3)).reshape(b, 2, 3, 3)
    rel_rot = rch[:, 1] @ rch[:, 0]
    sy = np.sqrt(rel_rot[:, 0, 0] ** 2 + rel_rot[:, 1, 0] ** 2)
    yang = np.arctan2(-rel_rot[:, 2, 0], sy).astype(f32) * f32(180.0 / np.pi)
    y = np.round(np.minimum(yang, f32(39.0))).astype(np.int32)
    neg = y < 0
    big = y < -39
    y = np.where(neg, np.where(big, 78, 39 - y), y)

    # device-side constant matrices
    # Smat [187, 3*VP] with columns ordered n-outer: col = n*VP + v
    # (padded to VP per plane: fp32r matmuls need even free counts)
    Smat = np.zeros((KX, 3 * VP), f32)
    Sv = Smat.reshape(KX, 3, VP)[:, :, :V]
    Sv[:150] = shapedirs.transpose(2, 1, 0)
    Sv[150] = v_template.T
    Sv[151:] = posedirs.reshape(36, V, 3).transpose(0, 2, 1)

    xaug = np.empty((B, KX), f32)
    xaug[:, :150] = betas
    xaug[:, 150] = 1.0
    xaug[:, 151:] = pose_feature

    # relT [60, B]: row = (c*4+n)*5 + j
    relT = rel.transpose(2, 3, 1, 0).reshape(60, b)                 # (c,n,j,b)
    relT = np.ascontiguousarray(relT.astype(f32))
    # rel3 [4, 128, B]: tile g holds cn=3g+t at partitions 32t+j (row-packing)
    rel3 = np.zeros((4, 128, b), f32)
    for g in range(4):
        for t in range(3):
            rel3[g, 32 * t:32 * t + 5] = relT[(3 * g + t) * 5:(3 * g + t) * 5 + 5]

    # three stacked copies: row-packed matmuls read the moving operand at
    # base partitions 0/32/64, so each row-group gets its own parallel load
    WTp = np.zeros((3 * J, VP), f32)
    for t in range(3):
        WTp[t * J:(t + 1) * J, :V] = lbs_weights.T

    return xaug, Smat, relT, rel3, WTp, y


# ------------------------------------------------------------- device build

def _build_device():
    import concourse.bacc as bacc
    import concourse.tile as tile
    from concourse import mybir

    R = mybir.dt.float32r
    F = mybir.dt.float32
    H = mybir.dt.bfloat16

    nc = bacc.Bacc("TRN2", target_bir_lowering=False, debug=False)
    d_x0 = nc.dram_tensor("x0", [KX0, B_LOC], H, kind="ExternalInput").ap()
    d_x1 = nc.dram_tensor("x1", [KX1, B_LOC], H, kind="ExternalInput").ap()
    d_s0 = nc.dram_tensor("s0", [KX0, 3 * VP], H, kind="ExternalInput").ap()
    d_s1 = nc.dram_tensor("s1", [KX1, 3 * VP], H, kind="ExternalInput").ap()
    d_rel = nc.dram_tensor("rel3", [4, 128, B_LOC], F, kind="ExternalInput").ap()
    d_wt = nc.dram_tensor("wt", [3 * J, VP], F, kind="ExternalInput").ap()
    d_out = nc.dram_tensor("verts", [B_LOC, 3 * V], F, kind="ExternalOutput").ap()

    with tile.TileContext(nc) as tc:
        with tc.tile_pool(name="const", bufs=1) as cpool, \
             tc.tile_pool(name="sstream", bufs=2) as spool, \
             tc.tile_pool(name="vp", bufs=1) as vpool, \
             tc.tile_pool(name="abuf", bufs=2) as apool, \
             tc.tile_pool(name="vout", bufs=2) as opool, \
             tc.tile_pool(name="tmp", bufs=2) as tpool, \
             tc.tile_pool(name="ps2", bufs=1, space="PSUM") as ps2, \
             tc.tile_pool(name="ps3", bufs=2, space="PSUM") as ps3:

            # ---- constants: 4 packed rel tiles (cn=3g+t at partitions 32t+j)
            wt = cpool.tile([69, VP], R, tag="wt")
            nc.scalar.dma_start(wt[0:J, :], d_wt[0:J, :].bitcast(R))
            nc.gpsimd.dma_start(wt[32:32 + J, :], d_wt[J:2 * J, :].bitcast(R))
            nc.scalar.dma_start(wt[64:64 + J, :], d_wt[2 * J:3 * J, :].bitcast(R))
            x0 = cpool.tile([KX0, B_LOC], H, tag="x0")
            x1 = cpool.tile([KX1, B_LOC], H, tag="x1")
            nc.gpsimd.dma_start(x0[:], d_x0[:])
            nc.gpsimd.dma_start(x1[:], d_x1[:])
            rel3 = []
            for g in range(4):
                t = cpool.tile([128, B_LOC], R, tag=f"rel3g{g}", name=f"rel3g{g}")
                eng = nc.scalar if g < 2 else nc.gpsimd
                eng.dma_start(t[:], d_rel[g].bitcast(R))
                rel3.append(t)

            vps = [[None] * 3 for _ in range(NFRAG)]

            for q in range(NFRAG):
                # ---- LBS transform matmuls first (only need tiny consts).
                # Quad qd covers planes {3g..3g+2}; the 3 K=5 matmuls sit in
                # row-groups 0/32/64 of the PE array and run concurrently.
                aq = apool.tile([B_LOC, 12 * VQ], H, tag="abuf")
                ap12 = aq[:].rearrange("p (t v) -> p t v", t=12)
                for k in range(CH_PER_Q):
                    v0 = (q * CH_PER_Q + k) * CHUNK
                    for g in range(4):
                        ps = ps3.tile([B_LOC, 3 * CHUNK], F, tag="ps3")
                        for t in range(3):
                            nc.tensor.matmul(
                                ps[:, t * CHUNK:(t + 1) * CHUNK],
                                rel3[g][32 * t:32 * t + 5, :],
                                wt[32 * t:32 * t + 5, v0:v0 + CHUNK],
                                start=True, stop=True)
                        dst = ap12[:, 3 * g:3 * g + 3,
                                   k * CHUNK:(k + 1) * CHUNK]
                        # f0 is latency-critical: alternate DVE/ACT evenly;
                        # steady state: DVE ~1.5 of 8 quads, rest ACT
                        if q == 0:
                            if g % 2 == 1:
                                nc.vector.tensor_copy(dst, ps[:])
                            else:
                                nc.scalar.copy(dst, ps[:])
                        elif g == 3 and k == 0:
                            nc.vector.tensor_copy(dst, ps[:])
                        elif g == 3 and k == 1:
                            nc.vector.tensor_copy(dst[:, 0:2, :],
                                                  ps[:, :2 * CHUNK])
                            nc.scalar.copy(dst[:, 2:3, :],
                                           ps[:, 2 * CHUNK:])
                        else:
                            nc.scalar.copy(dst, ps[:])

                # ---- blendshape+pose matmul for this fragment's vp planes
                for n in range(3):
                    s0t = spool.tile([KX0, VQ], H, tag="s0")
                    s1t = spool.tile([KX1, VQ], H, tag="s1")
                    base = n * VP + q * VQ
                    nc.sync.dma_start(s0t[:], d_s0[:, base:base + VQ])
                    nc.sync.dma_start(s1t[:], d_s1[:, base:base + VQ])
                    vpt = vpool.tile([B_LOC, VQ], H, tag=f"vp{n}q{q}",
                                     name=f"vp{n}q{q}")
                    ps = ps2.tile([B_LOC, VQ], F, tag="ps2")
                    for sub in range(0, VQ, 512):
                        nc.tensor.matmul(ps[:, sub:sub + 512], x0[:],
                                         s0t[:, sub:sub + 512],
                                         start=True, stop=False)
                        nc.tensor.matmul(ps[:, sub:sub + 512], x1[:],
                                         s1t[:, sub:sub + 512],
                                         start=False, stop=True)
                    nc.scalar.copy(vpt[:], ps[:])
                    vps[q][n] = vpt

                # ---- skinning apply (tree accumulation, flat plane APs)
                vo = opool.tile([B_LOC, 3 * VQ], F, tag="vout")
                voc = vo[:].rearrange("p (v c) -> p v c", c=3)
                vq = [vps[q][n][:] for n in range(3)]
                # last fragment: compute only the real (padded-to-even) width
                VA = min(VQ, ((3 * V - q * 3 * VQ) // 3 + 1) // 2 * 2)
                # f0 is latency-critical: apply per half-fragment so DVE can
                # start right after chunk 0's transform drains
                parts = [(0, VQ // 2), (VQ // 2, VQ)] if q == 0 else [(0, VA)]
                for plo, phi in parts:
                  t01s, t23s = [], []
                  for c in range(3):
                    sl = slice(plo, phi)
                    pa = tpool.tile([B_LOC, VQ], H, tag="pa")
                    pb = tpool.tile([B_LOC, VQ], H, tag="pb")
                    t01 = tpool.tile([B_LOC, VQ], H, tag=f"t01{c}",
                                     name=f"t01c{c}")
                    t23 = tpool.tile([B_LOC, VQ], H, tag=f"t23{c}",
                                     name=f"t23c{c}")
                    nc.vector.tensor_mul(pa[:, sl], ap12[:, 4 * c + 0, sl],
                                         vq[0][:, sl])
                    nc.vector.tensor_mul(pb[:, sl], ap12[:, 4 * c + 1, sl],
                                         vq[1][:, sl])
                    nc.vector.tensor_add(t01[:, sl], pa[:, sl], pb[:, sl])
                    p2 = tpool.tile([B_LOC, VQ], H, tag="pa", name="p2")
                    nc.vector.tensor_mul(p2[:, sl], ap12[:, 4 * c + 2, sl],
                                         vq[2][:, sl])
                    nc.vector.tensor_add(t23[:, sl], p2[:, sl],
                                         ap12[:, 4 * c + 3, sl])
                    t01s.append(t01)
                    t23s.append(t23)

                  # final adds + (v,c) interleave on the otherwise-idle
                  # gpsimd, by half so the store starts before the last final
                  nh = 1 if q == 0 else 2
                  HVQ = (phi - plo) // nh
                  for h in range(nh):
                    vsl = slice(plo + h * HVQ, min(plo + (h + 1) * HVQ, VA))
                    for c in range(3):
                        feng = nc.vector if (q == NFRAG - 1 and h == nh - 1) \
                            else nc.gpsimd
                        feng.tensor_add(voc[:, vsl, c],
                                        t01s[c][:, vsl], t23s[c][:, vsl])
                    vbase = q * 3 * VQ + 3 * (plo + h * HVQ)
                    vwid = min(3 * HVQ, max(0, 3 * V - vbase))
                    if vwid > 0:
                        nc.gpsimd.dma_start(
                            d_out[:, vbase:vbase + vwid],
                            vo[:, 3 * (plo + h * HVQ):
                               3 * (plo + h * HVQ) + vwid])

    nc.compile()
    return nc


def _get_nc():
    if "nc" not in _STATE:
        _STATE["nc"] = _build_device()
    return _STATE["nc"]


# -------------------------------------------------------------------- kernel

def kernel(shape_params, expression_params, pose_params, v_template, shapedirs,
           posedirs, J_regressor, lbs_weights, neck_pose, eye_pose, faces,
           lmk_faces_idx, lmk_bary_coords, dynamic_lmk_faces_idx,
           dynamic_lmk_bary_coords, full_lmk_faces_idx, full_lmk_bary_coords):
    from concourse.bass_utils import run_bass_kernel_spmd

    f32 = np.float32
    inputs = [np.asarray(a) for a in
              (shape_params, expression_params, pose_params, v_template,
               shapedirs, posedirs, J_regressor, lbs_weights, neck_pose,
               eye_pose)]
    xaug, Smat, relT, rel3, WTp, y = _host_prep(*inputs)

    s0 = np.ascontiguousarray(Smat[:KX0]).astype(BF16)
    s1 = np.ascontiguousarray(Smat[KX0:]).astype(BF16)
    xT = np.ascontiguousarray(xaug.T).astype(BF16)     # [187, B]

    core_ids = list(range(N_CORES))
    in_maps = []
    for c in core_ids:
        sl = slice(c * B_LOC, (c + 1) * B_LOC)
        in_maps.append({
            "x0": np.ascontiguousarray(xT[:KX0, sl]),
            "x1": np.ascontiguousarray(xT[KX0:, sl]),
            "s0": s0,
            "s1": s1,
            "rel3": np.ascontiguousarray(rel3[:, :, sl]),
            "wt": WTp,
        })

    nc = _get_nc()
    res = run_bass_kernel_spmd(nc, in_maps, core_ids)
    _STATE["last_result"] = res

    verts = np.concatenate([res.results[c]["verts"] for c in core_ids], 0)
    vertices = verts.reshape(B, V, 3).astype(f32, copy=False)

    # ---- landmarks on host (tiny gathers)
    faces = np.asarray(faces)
    lmk_faces_idx = np.asarray(lmk_faces_idx)
    lmk_bary_coords = np.asarray(lmk_bary_coords).astype(f32)
    dynamic_lmk_faces_idx = np.asarray(dynamic_lmk_faces_idx)
    dynamic_lmk_bary_coords = np.asarray(dynamic_lmk_bary_coords).astype(f32)
    full_lmk_faces_idx = np.asarray(full_lmk_faces_idx)
    full_lmk_bary_coords = np.asarray(full_lmk_bary_coords).astype(f32)

    bidx = np.arange(B)[:, None, None]

    dyn_tri = faces[dynamic_lmk_faces_idx[y]]            # [B,17,3]
    dyn_bary = dynamic_lmk_bary_coords[y]                # [B,17,3]
    lv = vertices[bidx, dyn_tri]                         # [B,17,3,3]
    lm_dyn = np.einsum("blfc,blf->blc", lv, dyn_bary)

    st_tri = faces[lmk_faces_idx]                        # [51,3]
    lv = vertices[:, st_tri]                             # [B,51,3,3]
    lm_st = np.einsum("blfc,lf->blc", lv, lmk_bary_coords)

    landmarks2d = np.concatenate([lm_dyn, lm_st], 1).astype(f32, copy=False)

    fl_tri = faces[full_lmk_faces_idx]                   # [68,3]
    lv = vertices[:, fl_tri]                             # [B,68,3,3]
    landmarks3d = np.einsum("blfc,lf->blc", lv,
                            full_lmk_bary_coords).astype(f32, copy=False)

    return vertices, landmarks2d, landmarks3d
